# revision 1
# baseline (speedup 1.0000x reference)
"""Converse2D (FFT-based closed-form deconvolution solve) on 8 Trainium2 cores.

Math reduction (s=2, H=W=128):
  The reference computes, per (b,c):
      out = real(ifft2_256( T[c] * tile2x2(fft2_128(x[b,c])) )) + bias[c]
  where T[c] is a transfer function depending only on (weight, lambda).
  Decimating the 256-point inverse FFT over output parity (a,b in {0,1}^2)
  gives   out[2m+a, 2n+b] = ifft2_128( X * T_ab[c] )[m,n]
  with X = fft2_128(x[b,c]) and T_ab precomputable on host.
  Pair-packing (out_a0 + i*out_a1 = ifft2(X * (T_a0 + i*T_a1))) halves the
  inverse-transform count: two complex 128x128 IFFTs per slice.

  On device (per slice, all matmuls bf16 with fp32 PSUM accumulation):
    stage1:  A1 = x^T F           (1 matmul, N=256)       [j,k]
    stage2:  X  = A1^T F          (2 matmuls, N=256)      [k,m]
    pointwise: Z_p = X * Tc_p[c]  (DVE, bf16, 2x mode)    p in {1,2}
    stageA:  B_p = Z_p^T G        (2 matmuls each, N=256) [l,m]
    stageB:  O_p = B_p^T G        (2 matmuls each, N=256) [m,n]
    out[2m+a, 2n+c] = Re/Im(O) interleaved + bias
  The "stationary-chaining" trick (previous stage's output is the next
  stage's stationary operand) makes every stage transpose-free.

Sharding: core k handles channels [8k, 8k+8), all 4 batches (transfer
functions are reused across the batch).
"""

import numpy as np
import ml_dtypes

import concourse.bass as bass
import concourse.bacc as bacc
import concourse.mybir as mybir
import concourse.tile as tile
from concourse.bass_utils import run_bass_kernel_spmd

BF16 = ml_dtypes.bfloat16

B, C, H, W, KK = 4, 64, 128, 128, 5
S = 2
HS, WS = H * S, W * S
NCORES = 8
CPC = C // NCORES  # channels per core


# ----------------------------------------------------------------------------
# host-side precompute of the pair-packed transfer functions
# ----------------------------------------------------------------------------
def _precompute_tc(weight: np.ndarray, lam: float) -> np.ndarray:
    """-> [C, 128, 1024] bf16: [T1r|T1i | T1i|T1r | T2r|T2i | T2i|T2r].

    The duplicated-pair layout lets every pointwise multiply be a fully
    contiguous bf16 tensor_tensor op (DVE 2x mode):
      P_k = [Xr|Xi] * block_k  for k in 0..3
    giving products [XrT1r|XiT1i | XrT1i|XiT1r | XrT2r|XiT2i | XrT2i|XiT2r].
    """
    psf = np.asarray(weight, np.float64)[0]  # [C,5,5]
    otf = np.zeros((C, HS, WS), np.complex128)
    otf[:, :KK, :KK] = psf
    otf = np.roll(otf, (-(KK // 2), -(KK // 2)), axis=(-2, -1))
    FB = np.fft.fft2(otf)
    FBC = np.conj(FB)
    F2B = (FB * FBC).real
    u = np.arange(HS)
    du = 1.0 + np.exp(-2j * np.pi * u / HS)
    G = FBC + lam * du[:, None] * du[None, :]

    def quad_mean(A):
        return 0.25 * (A[:, :H, :W] + A[:, H:, :W] + A[:, :H, W:] + A[:, H:, W:])

    M = quad_mean(FB * G) / (quad_mean(F2B) + lam)
    T = (G - FBC * np.tile(M, (1, 2, 2))) / lam

    ph = np.exp(2j * np.pi * np.arange(H) / HS)
    Tab = {}
    for a in range(2):
        for b in range(2):
            acc = np.zeros((C, H, W), np.complex128)
            for be in range(2):
                for ga in range(2):
                    acc += ((-1) ** (a * be + b * ga)) * T[
                        :, be * H : (be + 1) * H, ga * W : (ga + 1) * W
                    ]
            Tab[(a, b)] = 0.25 * (ph[:, None] ** a) * (ph[None, :] ** b) * acc
    scale = 1.0 / (H * W)  # fold ifft2_128 normalization
    Tc1 = (Tab[(0, 0)] + 1j * Tab[(0, 1)]) * scale
    Tc2 = (Tab[(1, 0)] + 1j * Tab[(1, 1)]) * scale
    t1r, t1i = Tc1.real, Tc1.imag
    t2r, t2i = Tc2.real, Tc2.imag
    out = np.concatenate(
        [t1r, t1i, t1i, t1r, t2r, t2i, t2i, t2r], axis=-1
    )  # [C,128,1024]
    return np.asarray(out, np.float32).astype(BF16)


# ----------------------------------------------------------------------------
# device program (built once, SPMD across 8 cores)
# ----------------------------------------------------------------------------
_CACHED_NC = None


def _build_nc():
    global _CACHED_NC
    if _CACHED_NC is not None:
        return _CACHED_NC

    f32 = mybir.dt.float32
    bf16 = mybir.dt.bfloat16

    idx = np.arange(H)
    Fc = np.exp(-2j * np.pi * np.outer(idx, idx) / H)
    Fr = Fc.real.astype(np.float32)
    Fi = Fc.imag.astype(np.float32)
    # forward rhs:  CF = [Fr | Fi],  CF2 = [-Fi | Fr]
    # inverse (G = conj(F) = Fr - i*Fi): CG = [Fr | -Fi], CG2 = [Fi | Fr]
    CF = np.concatenate([Fr, Fi], axis=1).astype(BF16)
    CF2 = np.concatenate([-Fi, Fr], axis=1).astype(BF16)
    CG = np.concatenate([Fr, -Fi], axis=1).astype(BF16)
    CG2 = np.concatenate([Fi, Fr], axis=1).astype(BF16)

    nc = bacc.Bacc()
    xs_ext = nc.dram_tensor("xs", [CPC, H, B * W], bf16, kind="ExternalInput")
    tc_ext = nc.dram_tensor("tc", [CPC, H, 8 * W], bf16, kind="ExternalInput")
    bias_ext = nc.dram_tensor("bias", [128, CPC], f32, kind="ExternalInput")
    out_ext = nc.dram_tensor("out", [CPC, B, H, 4 * W], f32, kind="ExternalOutput")

    cf_d = nc.inline_tensor(CF, "cf_d")
    cf2_d = nc.inline_tensor(CF2, "cf2_d")
    cg_d = nc.inline_tensor(CG, "cg_d")
    cg2_d = nc.inline_tensor(CG2, "cg2_d")

    with tile.TileContext(nc) as tc:
        from contextlib import ExitStack

        with ExitStack() as ctx:
            consts = ctx.enter_context(tc.tile_pool(name="consts", bufs=1))
            # load pools sized so slots are never reused: load DMAs then carry
            # no descriptor-level sync waits (HW limit), only completion incs
            tpool = ctx.enter_context(tc.tile_pool(name="tpool", bufs=CPC))
            xpool = ctx.enter_context(tc.tile_pool(name="xpool", bufs=CPC))
            a1pool = ctx.enter_context(tc.tile_pool(name="a1pool", bufs=4))
            xspool = ctx.enter_context(tc.tile_pool(name="xspool", bufs=4))
            ppool = ctx.enter_context(tc.tile_pool(name="ppool", bufs=3))
            zpool = ctx.enter_context(tc.tile_pool(name="zpool", bufs=4))
            bspool = ctx.enter_context(tc.tile_pool(name="bspool", bufs=4))
            # opool never reuses a slot: the final-copy then carries no
            # DMA-release wait (saves one event-semaphore per slice on ACT)
            opool = ctx.enter_context(tc.tile_pool(name="opool", bufs=CPC * B))
            pA = ctx.enter_context(tc.tile_pool(name="pA", bufs=2, space="PSUM"))
            pX = ctx.enter_context(tc.tile_pool(name="pX", bufs=2, space="PSUM"))
            pB = ctx.enter_context(tc.tile_pool(name="pB", bufs=2, space="PSUM"))
            pO = ctx.enter_context(tc.tile_pool(name="pO", bufs=2, space="PSUM"))

            cf = consts.tile([128, 256], bf16, tag="cf")
            cf2 = consts.tile([128, 256], bf16, tag="cf2")
            cg = consts.tile([128, 256], bf16, tag="cg")
            cg2 = consts.tile([128, 256], bf16, tag="cg2")
            bias_t = consts.tile([128, CPC], f32, tag="bias")
            nc.sync.dma_start(cf[:], cf_d[:])
            nc.sync.dma_start(cf2[:], cf2_d[:])
            nc.sync.dma_start(cg[:], cg_d[:])
            nc.sync.dma_start(cg2[:], cg2_d[:])
            nc.sync.dma_start(bias_t[:], bias_ext[:])

            for ci in range(CPC):
                # all 4 batches of this channel in one DMA: [128, 4*128]
                xt4 = xpool.tile([128, B * W], bf16)
                nc.gpsimd.dma_start(xt4[:], xs_ext[ci])
                tt = tpool.tile([128, 1024], bf16)
                nc.gpsimd.dma_start(tt[:], tc_ext[ci])
                bias_ap = bias_t[:, ci : ci + 1]

                for bi in range(B):
                    xt = xt4[:, bi * W : (bi + 1) * W]

                    # stage1: A1 = x^T F ; stage2: X = A1^T F (shared psum bank)
                    pa = pA.tile([128, 256], f32)
                    nc.tensor.matmul(pa[:], xt, cf[:], start=True, stop=True)
                    a1 = a1pool.tile([128, 256], bf16)
                    nc.scalar.copy(a1[:], pa[:])
                    px = pX.tile([128, 256], f32)
                    nc.tensor.matmul(px[:], a1[:, 0:128], cf[:], start=True, stop=False)
                    nc.tensor.matmul(
                        px[:], a1[:, 128:256], cf2[:], start=False, stop=True
                    )
                    xsb = xspool.tile([128, 256], bf16)
                    nc.vector.tensor_copy(xsb[:], px[:])

                    # pointwise products, all contiguous bf16 (DVE 2x):
                    # P = [XrT1r|XiT1i | XrT1i|XiT1r | XrT2r|XiT2i | XrT2i|XiT2r]
                    pp = ppool.tile([128, 1024], bf16)
                    nc.vector.tensor_mul(
                        pp[:].rearrange("p (r f) -> p r f", r=4),
                        xsb[:].unsqueeze(1).broadcast_to((128, 4, 256)),
                        tt[:].rearrange("p (r f) -> p r f", r=4),
                    )
                    # combines: Z = [Z1r | Z1i | Z2r | Z2i]
                    z = zpool.tile([128, 512], bf16)
                    z4 = z[:].rearrange("p (a c f) -> p a c f", a=2, c=2)
                    p8 = pp[:].rearrange(
                        "p (k2 k1 t f) -> p k2 k1 t f", k2=2, k1=2, t=2
                    )
                    nc.vector.tensor_sub(
                        z4[:, :, 0, :], p8[:, :, 0, 0, :], p8[:, :, 0, 1, :]
                    )
                    nc.vector.tensor_add(
                        z4[:, :, 1, :], p8[:, :, 1, 0, :], p8[:, :, 1, 1, :]
                    )

                    # stageA: B_p = Z_p^T G  (both pairs into one psum bank)
                    pb = pB.tile([128, 512], f32)
                    nc.tensor.matmul(
                        pb[:, 0:256], z[:, 0:128], cg[:], start=True, stop=False
                    )
                    nc.tensor.matmul(
                        pb[:, 0:256], z[:, 128:256], cg2[:], start=False, stop=True
                    )
                    nc.tensor.matmul(
                        pb[:, 256:512], z[:, 256:384], cg[:], start=True, stop=False
                    )
                    nc.tensor.matmul(
                        pb[:, 256:512], z[:, 384:512], cg2[:], start=False, stop=True
                    )
                    bs = bspool.tile([128, 512], bf16)
                    nc.scalar.copy(bs[:], pb[:])

                    # stageB: O_p = B_p^T G
                    po = pO.tile([128, 512], f32)
                    nc.tensor.matmul(
                        po[:, 0:256], bs[:, 0:128], cg[:], start=True, stop=False
                    )
                    nc.tensor.matmul(
                        po[:, 0:256], bs[:, 128:256], cg2[:], start=False, stop=True
                    )
                    nc.tensor.matmul(
                        po[:, 256:512], bs[:, 256:384], cg[:], start=True, stop=False
                    )
                    nc.tensor.matmul(
                        po[:, 256:512], bs[:, 384:512], cg2[:], start=False, stop=True
                    )

                    # final: interleave Re/Im into output row-pairs, add bias
                    # po blocks: (pair a, comp c) at offset 128*(2a+c)
                    # ot layout:  ot[m, 256a + 2q + c] = out[2m+a, 2q+c]
                    ot = opool.tile([128, 512], f32)
                    src = po[:].rearrange("p (a c q) -> p a c q", a=2, c=2)
                    dst = ot[:].rearrange("p (a q c) -> p a c q", a=2, q=128, c=2)
                    nc.scalar.add(dst, src, bias_ap)

                    nc.sync.dma_start(out_ext[ci, bi], ot[:])

    nc.finalize()  # Bacc.finalize runs the pass pipeline (multi-wait splitting etc.)
    _CACHED_NC = nc
    return nc


# ----------------------------------------------------------------------------
# public entry point
# ----------------------------------------------------------------------------
def _run(x, weight, bias, lambda_reg, trace=False, trace_kwargs=None):
    x = np.asarray(x)
    weight = np.asarray(weight)
    bias = np.asarray(bias)
    lam = float(np.asarray(lambda_reg).reshape(()))

    tc_all = _precompute_tc(weight, lam)  # [C,128,1024] bf16
    bias_vals = np.asarray(bias, np.float32).reshape(C)
    x_bf = np.asarray(x, np.float32).astype(BF16)

    in_maps = []
    for k in range(NCORES):
        c0, c1 = k * CPC, (k + 1) * CPC
        in_maps.append(
            {
                "xs": np.ascontiguousarray(
                    x_bf[:, c0:c1].transpose(1, 2, 0, 3).reshape(CPC, H, B * W)
                ),
                "tc": np.ascontiguousarray(tc_all[c0:c1]),
                "bias": np.ascontiguousarray(
                    np.broadcast_to(bias_vals[c0:c1][None, :], (128, CPC))
                ),
            }
        )

    nc = _build_nc()
    kwargs = {}
    if trace:
        kwargs["trace"] = True
        if trace_kwargs:
            kwargs.update(trace_kwargs)
    res = run_bass_kernel_spmd(nc, in_maps, list(range(NCORES)), **kwargs)

    out = np.empty((B, C, HS, WS), np.float32)
    for k in range(NCORES):
        c0, c1 = k * CPC, (k + 1) * CPC
        oc = res.results[k]["out"]  # [CPC, B, 128, 512]
        out[:, c0:c1] = (
            oc.reshape(CPC, B, H, 2, WS).transpose(1, 0, 2, 3, 4).reshape(B, CPC, HS, WS)
        )
    return out, res


def kernel(x, weight, bias, lambda_reg):
    out, _ = _run(x, weight, bias, lambda_reg)
    return out

